# revision 58
# baseline (speedup 1.0000x reference)
"""DeepCross kernel for 8x TRN2 NeuronCores.

Math: the cross-network keeps temp = x0 * f with f a per-row scalar, so the
whole model collapses to G = x0 @ [cross_w | w1 | wf_x0]  ([B, 37]) plus a
tiny per-row tail:
    g = G[:, :4]; p1 = G[:, 4:36]; q = G[:, 36:37]
    f1 = 1 + g0 + b0; f2 = f1*g1 + b1; f3 = f2*(1+g2) + b2; f4 = f3*g3 + b3
    h1 = relu(f4 * p1); h2 = relu(h1 @ w2); out = sigmoid(h2 @ wf_h + q + bf)

Device strategy (data-parallel over batch, 1024 rows/core):
  - emb table packed to bf16 row-PAIRS [50000, 256B]; dma_gather with
    idx = x//2 stored int16-WRAPPED (values >= 32768 go negative; the HW
    sign-extends and the address wraps 16.77MB below the in_ap base, so the
    table tensor is 2x sized with the high pairs mirrored into the low half
    and in_ap based at +16.77MB).  One descriptor per lookup, 256B each.
  - gathers run on all 4 SWDGE queues (num_swdge_queues=4) so all 4 GPSIMD
    Q7 core-pairs emit descriptors concurrently; Q7 descriptor emission
    (~7ns/lookup/pair) is the kernel's hard bottleneck.  The engine blocks
    on each queue-0 gather until its emission ends, so the FIRST job is
    tiny (2 slots) to let the other queues start immediately (saves a full
    ~30us warmup wave).
  - 1-of-2 sub-row select: one base copy + one predicated copy on fp32
    bitcast views (halves DVE/ACT element counts).
  - PE-transpose 128x128 chunks, accumulate G^T [37, 128] per subtile on PE.
  - One batched tail after all gathers (mid-stream DVE tail ops run ~10x
    slow under SWDGE activity and delay the selects): f-recurrence on DVE,
    h1@w2 on PE, sigmoid on ACT; out_col [128, 8] DMA'd per core and
    untransposed on host.
"""
import sys
sys.path.insert(0, '/opt/trn_rl_repo')
import os
import numpy as np
import ml_dtypes

from concourse import bass, mybir
import concourse.tile as tile
from concourse import bacc, library_config
from concourse.bass_utils import run_bass_kernel_spmd
from concourse.masks import make_identity
from concourse.tile import add_dep_helper

BF16 = ml_dtypes.bfloat16

B, T, E = 8192, 128, 64
V = 100000
D = T * E                 # 8192
L = 4
H1, H2 = 32, 16
NCORES = 8
BC = B // NCORES          # 1024 batch rows per core
NSUB = BC // 128          # 8 subtiles of 128 rows
NPAD = 16                 # trailing dummy idxs (defeat trailing-neg trim)
PE_EL = 128               # bf16 elements per gathered pair row (256 B)
PAIRS = V // 2            # 50000 pair rows
TBL_N = 131072            # table tensor rows (2x 65536 for the wrap trick)
NCHUNK = D // 128         # 64 d-chunks per subtile
NW = L + H1 + 1           # 37 fused weight columns


def _groups():
    """Flat gather-group schedule shared by host prep and program build.
    The very first group is tiny (2 slots): the GPSIMD engine blocks on each
    queue-0 gather until its emission ends, so a small first job lets the
    other three queues' first gathers dispatch almost immediately instead of
    one full emission later.  Subs 0-5: 32-slot groups; subs 6-7: 16-slot
    groups (shorter final waves so the last subtiles' post-processing
    overlaps better)."""
    out = []
    icol = mcol = 0

    def add(sub, slot0, gsz):
        nonlocal icol, mcol
        nidx = 128 * gsz + NPAD
        out.append(dict(sub=sub, slot0=slot0, nslots=gsz,
                        nidx=nidx, nblk=(nidx + 127) // 128,
                        icol=icol, mcol=mcol))
        icol += nidx // 16
        mcol += gsz

    add(0, 0, 2)
    add(0, 2, 62)
    add(0, 64, 64)
    for sub in range(1, NSUB):
        for h in range(2):
            add(sub, h * 64, 64)
    return out


GROUPS = _groups()
IDX_COLS = sum(g['nidx'] // 16 for g in GROUPS)
MSK_COLS = sum(g['nslots'] for g in GROUPS)

_PROGRAM = None
KMODE = os.environ.get('KMODE', 'full')
NSWQ = int(os.environ.get('NSWQ', '4'))     # SWDGE queues (1..4)


def _build_program():
    f32 = mybir.dt.float32
    bf16 = mybir.dt.bfloat16
    nc = bacc.Bacc("TRN2", target_bir_lowering=False, debug=False,
                   num_devices=NCORES, dynamic_dma_scratch_size=32768,
                   num_swdge_queues=NSWQ)

    tblq = nc.dram_tensor("tblq", [TBL_N, PE_EL], bf16, kind="ExternalInput")
    xidx = nc.dram_tensor("xidx", [128, IDX_COLS], mybir.dt.int16,
                          kind="ExternalInput")
    xmask = nc.dram_tensor("xmask", [128, MSK_COLS], mybir.dt.uint8,
                           kind="ExternalInput")
    wbd = nc.dram_tensor("wb", [128, NCHUNK * NW], bf16, kind="ExternalInput")
    w2d = nc.dram_tensor("w2m", [H1, H2], f32, kind="ExternalInput")
    # packed tail constants: [cb(4) | b1(32) | b2(16) | bf(1) | w2T(512) | wfh(16)]
    NPK = L + H1 + H2 + 1 + H1 * H2 + H2
    packd = nc.dram_tensor("packv", [1, NPK], f32, kind="ExternalInput")
    outd = nc.dram_tensor("out", [128, NSUB], f32, kind="ExternalOutput")

    tbl_ap = tblq.ap()[TBL_N // 2:TBL_N, :]   # base at +16.77MB

    AF = mybir.ActivationFunctionType
    OP = mybir.AluOpType

    with tile.TileContext(nc) as tc:
        with (
            tc.tile_pool(name="const", bufs=1) as cpool,
            tc.tile_pool(name="io", bufs=3) as iopool,
            tc.tile_pool(name="quad", bufs=6) as qpool,
            tc.tile_pool(name="x0c", bufs=4) as xpool,
            tc.tile_pool(name="xt", bufs=4) as xtpool,
            tc.tile_pool(name="tail", bufs=2) as sp,
            tc.tile_pool(name="ptp", bufs=4, space="PSUM") as ptpool,
            tc.tile_pool(name="pgt", bufs=2, space="PSUM") as pgpool,
            tc.tile_pool(name="pts", bufs=2, space="PSUM") as pspool,
        ):
            # identity builds use gpsimd ops — issue before load_library so
            # they run during the preamble instead of after LOAD_LIB
            ident = cpool.tile([128, 128], bf16)
            make_identity(nc, ident[:])
            ident32 = cpool.tile([128, 128], f32)
            make_identity(nc, ident32[:])
            nc.gpsimd.load_library(library_config.mlp)

            # idx upload split (separate tiles so wave-0 gathers only dep on
            # the small first DMA)
            ACOLS = sum(g['nidx'] // 16 for g in GROUPS[:4])
            idx_a = cpool.tile([128, ACOLS], mybir.dt.int16, tag="idx_a")
            nc.sync.dma_start(out=idx_a[:], in_=xidx.ap()[:, 0:ACOLS])
            idx_b = cpool.tile([128, IDX_COLS - ACOLS], mybir.dt.int16,
                               tag="idx_b")
            nc.sync.dma_start(out=idx_b[:], in_=xidx.ap()[:, ACOLS:])

            def idx_of(gr):
                c0, c1 = gr['icol'], gr['icol'] + gr['nidx'] // 16
                if c1 <= ACOLS:
                    return idx_a[:, c0:c1]
                return idx_b[:, c0 - ACOLS:c1 - ACOLS]
            msk_all = cpool.tile([128, MSK_COLS], mybir.dt.uint8,
                                 tag="msk_all")
            nc.sync.dma_start(out=msk_all[:], in_=xmask.ap())

            wb_t = cpool.tile([128, NCHUNK * NW], bf16)
            nc.sync.dma_start(out=wb_t[:], in_=wbd.ap())
            w2_t = cpool.tile([H1, H2], f32)
            nc.sync.dma_start(out=w2_t[:], in_=w2d.ap())
            # broadcast packed tail constants to all 128 partitions via a
            # k=1 matmul with a ones column.
            pack_t = cpool.tile([1, NPK], f32)
            nc.sync.dma_start(out=pack_t[:], in_=packd.ap())
            ones_r = cpool.tile([1, 128], f32)
            nc.vector.memset(ones_r[:], 1.0)
            packb = cpool.tile([128, NPK], f32)
            packb_mm = None
            for off in range(0, NPK, 512):
                w = min(512, NPK - off)
                pb_p = pspool.tile([128, 512], f32, tag="tps")
                packb_mm = nc.tensor.matmul(out=pb_p[:, 0:w], lhsT=ones_r[:],
                                            rhs=pack_t[0:1, off:off + w],
                                            start=True, stop=True)
                nc.vector.tensor_copy(out=packb[:, off:off + w],
                                      in_=pb_p[:, 0:w])
            prev_tail_pe0 = packb_mm.ins
            cbb = packb[:, 0:L]
            b1b = packb[:, L:L + H1]
            b2b = packb[:, L + H1:L + H1 + H2]
            bfb = packb[:, L + H1 + H2:L + H1 + H2 + 1]
            OW2 = L + H1 + H2 + 1
            w2b = packb[:, OW2:OW2 + H1 * H2]        # w2T j-major [16, 32]
            wfhb = packb[:, OW2 + H1 * H2:NPK]       # [16]
            out_col = None
            if KMODE != "notail":
                out_col = cpool.tile([128, NSUB], f32, tag="out_col")
            gts_all = cpool.tile([NW, BC], f32, tag="gts_all")

            gtt_all = None
            if KMODE != "notail":
                gtt_all = cpool.tile([128, NSUB * NW], f32, tag="gtt_all")

            def gtt_for(s):
                """Transpose G^T [37,128] -> G [128,37] into gtt_all.  PE+ACT
                only, so it is safe to issue mid-stream right after each
                stash (no DVE-under-SWDGE slowdown)."""
                gtt_p = pspool.tile([128, 64], f32, tag="tps")
                nc.tensor.transpose(
                    out=gtt_p[:, 0:NW],
                    in_=gts_all[:, s * 128:(s + 1) * 128],
                    identity=ident32[0:NW, 0:NW])
                nc.scalar.copy(out=gtt_all[:, s * NW:(s + 1) * NW],
                               in_=gtt_p[:, 0:NW])

            def tail_half(s0, ns):
                """Tail math for subtiles [s0, s0+ns): f-recurrence on DVE,
                h1@w2 on PE, sigmoid into out_col[:, s0:s0+ns]."""
                gv = gtt_all[:, s0 * NW:(s0 + ns) * NW].rearrange(
                    "p (s w) -> p s w", w=NW)

                f1 = cpool.tile([128, ns], f32, tag=f"f1_{s0}")
                nc.vector.tensor_scalar(out=f1[:], in0=gv[:, :, 0:1],
                                        scalar1=cbb[:, 0:1], scalar2=1.0,
                                        op0=OP.add, op1=OP.add)
                f2 = cpool.tile([128, ns], f32, tag=f"f2_{s0}")
                nc.vector.tensor_tensor(out=f2[:], in0=f1[:].rearrange(
                    "p (s one) -> p s one", one=1), in1=gv[:, :, 1:2],
                    op=OP.mult)
                nc.vector.tensor_scalar(out=f2[:], in0=f2[:],
                                        scalar1=cbb[:, 1:2], scalar2=None,
                                        op0=OP.add)
                u3 = cpool.tile([128, ns], f32, tag=f"u3_{s0}")
                nc.vector.tensor_scalar(out=u3[:], in0=gv[:, :, 2:3],
                                        scalar1=1.0, scalar2=None, op0=OP.add)
                f3 = cpool.tile([128, ns], f32, tag=f"f3_{s0}")
                nc.vector.tensor_tensor(out=f3[:], in0=f2[:], in1=u3[:],
                                        op=OP.mult)
                nc.vector.tensor_scalar(out=f3[:], in0=f3[:],
                                        scalar1=cbb[:, 2:3], scalar2=None,
                                        op0=OP.add)
                f4 = cpool.tile([128, ns], f32, tag=f"f4_{s0}")
                nc.vector.tensor_tensor(out=f4[:], in0=f3[:].rearrange(
                    "p (s one) -> p s one", one=1), in1=gv[:, :, 3:4],
                    op=OP.mult)
                nc.vector.tensor_scalar(out=f4[:], in0=f4[:],
                                        scalar1=cbb[:, 3:4], scalar2=None,
                                        op0=OP.add)

                # h1 = relu(f4 * p1 + b1)  [128, ns, H1]
                h1_all = cpool.tile([128, ns * H1], f32, tag=f"h1_{s0}")
                h1v = h1_all[:].rearrange("p (s h) -> p s h", h=H1)
                nc.vector.tensor_tensor(
                    out=h1v, in0=gv[:, :, L:L + H1],
                    in1=f4[:].rearrange("p (s one) -> p s one", one=1)
                        .to_broadcast([128, ns, H1]),
                    op=OP.mult)
                nc.vector.tensor_tensor(
                    out=h1v, in0=h1v,
                    in1=b1b.rearrange("p (one h) -> p one h", one=1)
                        .to_broadcast([128, ns, H1]),
                    op=OP.add)
                nc.scalar.activation(out=h1_all[:], in_=h1_all[:], func=AF.Relu)

                # h2 = relu(h1 @ w2 + b2) via PE (idle at tail time):
                # transpose h1 into (s,h)-on-partitions halves, 4 subs each
                h2_all = cpool.tile([128, ns * H2], f32, tag=f"h2_{s0}")
                h2v = h2_all[:].rearrange("p (s j) -> p s j", j=H2)
                for q4 in range((ns + 3) // 4):
                    nsq = min(4, ns - q4 * 4)
                    h1T_p = pspool.tile([128, 128], f32, tag="tps")
                    nc.tensor.transpose(
                        out=h1T_p[0:nsq * H1, :],
                        in_=h1_all[:, q4 * 4 * H1:(q4 * 4 + nsq) * H1],
                        identity=ident32[:])
                    h1T = cpool.tile([128, 128], f32, tag=f"h1T_{s0}_{q4}")
                    nc.scalar.copy(out=h1T[0:nsq * H1, :],
                                   in_=h1T_p[0:nsq * H1, :])
                    for j in range(nsq):
                        sl = q4 * 4 + j
                        h1Tj = cpool.tile([H1, 128], f32,
                                          tag=f"h1Tj_{s0}_{sl}")
                        nc.scalar.copy(out=h1Tj[:],
                                       in_=h1T[j * H1:(j + 1) * H1, :])
                        h2p = pspool.tile([128, H2], f32, tag="tps")
                        nc.tensor.matmul(out=h2p[:], lhsT=h1Tj[:],
                                         rhs=w2_t[:], start=True, stop=True)
                        nc.scalar.copy(out=h2_all[:, sl * H2:(sl + 1) * H2],
                                       in_=h2p[:])
                nc.vector.tensor_tensor(
                    out=h2v, in0=h2v,
                    in1=b2b.rearrange("p (one h) -> p one h", one=1)
                        .to_broadcast([128, ns, H2]),
                    op=OP.add)
                nc.scalar.activation(out=h2_all[:], in_=h2_all[:], func=AF.Relu)

                # z = h2 @ wf_h ; out = sigmoid(z + q + bf)
                zt = cpool.tile([128, ns * H2], f32, tag=f"zt_{s0}")
                ztv = zt[:].rearrange("p (s j) -> p s j", j=H2)
                nc.vector.tensor_tensor(
                    out=ztv, in0=h2v,
                    in1=wfhb.rearrange("p (one j) -> p one j", one=1)
                        .to_broadcast([128, ns, H2]),
                    op=OP.mult)
                z_all = cpool.tile([128, ns], f32, tag=f"z_{s0}")
                nc.vector.tensor_reduce(
                    out=z_all[:].rearrange("p (s one) -> p s one", one=1),
                    in_=ztv, axis=mybir.AxisListType.X, op=OP.add)
                nc.vector.tensor_tensor(out=z_all[:], in0=z_all[:].rearrange(
                    "p (s one) -> p s one", one=1), in1=gv[:, :, NW - 1:NW],
                    op=OP.add)
                nc.scalar.activation(out=out_col[:, s0:s0 + ns], in_=z_all[:],
                                     func=AF.Sigmoid,
                                     bias=bfb[:, 0:1], scale=1.0)

            prev_tail_pe = prev_tail_pe0
            MAXBLK = max(g['nblk'] for g in GROUPS)
            gt = None
            cur_sub = -1
            for jidx, gr in enumerate(GROUPS):
                sub, slot0, nsl = gr['sub'], gr['slot0'], gr['nslots']
                if sub != cur_sub:
                    gt = pgpool.tile([NW, 128], f32, tag="gt",
                                     name=f"gt{sub}")
                    cur_sub = sub
                idx_t = idx_of(gr)
                msk_t = msk_all[:, gr['mcol']:gr['mcol'] + nsl]

                quad = qpool.tile([128, MAXBLK * PE_EL], bf16, tag="quad")
                qview = quad[:].rearrange("p (s e) -> p s e", e=PE_EL)
                nc.gpsimd.dma_gather(
                    out_ap=qview[:, 0:gr['nblk'], :],
                    in_ap=tbl_ap,
                    idxs_ap=idx_t[:],
                    num_idxs=gr['nidx'],
                    num_idxs_reg=gr['nidx'],
                    elem_size=PE_EL,
                    single_packet=False,
                    queue_num=jidx % NSWQ,
                )

                x0c = xpool.tile([128, 64 * 64], bf16, tag="x0c")
                qv32 = quad[:].bitcast(f32).rearrange("p (s e) -> p s e",
                                                      e=PE_EL // 2)
                xv32 = x0c[:].bitcast(f32).rearrange("p (s e) -> p s e",
                                                     e=32)
                HG = nsl // 2

                def _mk(lo, hi):
                    m = msk_t[:, lo:hi]
                    m = m.rearrange("p (s one) -> p s one", one=1)
                    return m.to_broadcast([128, hi - lo, 32])

                for lo in (0, HG):
                    hi = lo + HG
                    nc.scalar.copy(out=xv32[:, lo:hi, :],
                                   in_=qv32[:, lo:hi, 0:32])
                    nc.vector.copy_predicated(out=xv32[:, lo:hi, :],
                                              mask=_mk(lo, hi),
                                              data=qv32[:, lo:hi, 32:64])
                chunks = nsl // 2
                for c0 in range(0, chunks, 4):
                    blkn = min(4, chunks - c0)
                    tp = ptpool.tile([128, 512], bf16, tag="tp")
                    for j in range(blkn):
                        c2 = c0 + j
                        nc.tensor.transpose(
                            out=tp[:, j * 128:(j + 1) * 128],
                            in_=x0c[:, c2 * 128:(c2 + 1) * 128],
                            identity=ident[:],
                        )
                    xt = xtpool.tile([128, 512], bf16, tag="xt")
                    nc.scalar.copy(out=xt[:, 0:blkn * 128],
                                   in_=tp[:, 0:blkn * 128])
                    for j in range(blkn):
                        cd = slot0 // 2 + c0 + j
                        mm = nc.tensor.matmul(
                            out=gt[:],
                            lhsT=wb_t[:, cd * NW:(cd + 1) * NW],
                            rhs=xt[:, j * 128:(j + 1) * 128],
                            start=(cd == 0),
                            stop=(cd == NCHUNK - 1),
                        )
                        if cd == 0 and sub == 0 and prev_tail_pe is not None:
                            add_dep_helper(mm.ins, prev_tail_pe,
                                           reason="packb before accum groups")

                if slot0 + nsl == 128:
                    # stash G^T for the batched tail
                    nc.scalar.copy(out=gts_all[:, sub * 128:(sub + 1) * 128],
                                   in_=gt[:])
                    if KMODE != "notail":
                        gtt_for(sub)

            if KMODE == "notail":
                nc.sync.dma_start(out=outd.ap(), in_=wb_t[:, 0:NSUB])
            else:
                # single post-loop tail: mid-stream DVE injections run ~10x
                # slow under SWDGE activity and delay the selects
                tail_half(0, NSUB)
                nc.sync.dma_start(out=outd.ap(), in_=out_col[:])

    nc.compile()
    return nc


def _get_program():
    global _PROGRAM
    if _PROGRAM is None:
        _PROGRAM = _build_program()
    return _PROGRAM


def _host_prep(x, emb, cross_w, cross_b, w1, b1, w2, b2, wf, bf):
    x = np.asarray(x)
    emb = np.ascontiguousarray(np.asarray(emb, dtype=np.float32))
    cross_w = np.asarray(cross_w, dtype=np.float32)
    cross_b = np.asarray(cross_b, dtype=np.float32)
    w1 = np.asarray(w1, dtype=np.float32)
    w2 = np.asarray(w2, dtype=np.float32)
    b1 = np.asarray(b1, dtype=np.float32)
    b2 = np.asarray(b2, dtype=np.float32)
    wf = np.asarray(wf, dtype=np.float32)
    bf = np.asarray(bf, dtype=np.float32)

    # pair table with the int16-wrap mirror: real pairs at [65536, 65536+50000),
    # pairs >= 32768 mirrored at their raw index for wrapped (negative) idxs.
    pe = emb.astype(BF16).reshape(PAIRS, PE_EL)
    tbl = np.zeros((TBL_N, PE_EL), dtype=BF16)
    tbl[TBL_N // 2:TBL_N // 2 + PAIRS] = pe
    tbl[32768:PAIRS] = pe[32768:PAIRS]

    wbig = np.concatenate([cross_w[:, :, 0].T, w1, wf[H2:, :]], axis=1)  # [D, 37]
    wb_np = np.ascontiguousarray(
        wbig.reshape(NCHUNK, 128, NW).transpose(1, 0, 2).reshape(128, NCHUNK * NW)
    ).astype(BF16)

    # [cb(4) | b1(32) | b2(16) | bf(1) | w2T j-major (512) | wfh (16)]
    packv = np.concatenate([
        cross_b.reshape(-1), b1.reshape(-1), b2.reshape(-1), bf.reshape(-1),
        w2.T.reshape(-1), wf[:H2, 0].reshape(-1),
    ]).astype(np.float32).reshape(1, -1)

    shared = {
        "tblq": tbl,
        "wb": wb_np,
        "packv": packv,
        "w2m": w2,
    }

    in_maps = []
    for c in range(NCORES):
        xc = x[c * BC:(c + 1) * BC].astype(np.int64)
        xq = (xc // 2).astype(np.int32)           # pair idx, wraps to int16
        xr = (xc % 2).astype(np.uint8)
        idx_np = np.empty((128, IDX_COLS), dtype=np.int16)
        msk_np = np.empty((128, MSK_COLS), dtype=np.uint8)
        for gr in GROUPS:
            s, slot0, nsl = gr['sub'], gr['slot0'], gr['nslots']
            blk = xq[s * 128:(s + 1) * 128, slot0:slot0 + nsl]  # [128b, nsl]
            lst = np.concatenate([blk.T.reshape(-1),            # i = t*128+b
                                  np.zeros(NPAD, dtype=np.int32)])
            cw = gr['nidx'] // 16
            idx_np[:, gr['icol']:gr['icol'] + cw] = np.tile(
                lst.reshape(cw, 16).T.astype(np.int16), (8, 1))
            msk_np[:, gr['mcol']:gr['mcol'] + nsl] = \
                xr[s * 128:(s + 1) * 128, slot0:slot0 + nsl]
        m = dict(shared)
        m["xidx"] = idx_np
        m["xmask"] = msk_np
        in_maps.append(m)
    return in_maps


def _ensure_ntff_hook():
    """The image's antenv lacks axon_hooks; synthesize it so
    run_bass_kernel_spmd(trace=True) can NTFF-profile via the axon .so."""
    import types
    if 'antenv.axon_hooks' in sys.modules:
        return
    import antenv
    mod = types.ModuleType('antenv.axon_hooks')
    _state = {'hook': None}
    def set_axon_ntff_profile_hook(h):
        _state['hook'] = h
    def get_axon_ntff_profile_hook():
        if _state['hook'] is None:
            try:
                from trn_agent_boot.trn_boot import _ntff_profile_via_ctypes
                _state['hook'] = _ntff_profile_via_ctypes('/opt/axon/libaxon_pjrt.so')
            except Exception:
                return None
        return _state['hook']
    mod.set_axon_ntff_profile_hook = set_axon_ntff_profile_hook
    mod.get_axon_ntff_profile_hook = get_axon_ntff_profile_hook
    sys.modules['antenv.axon_hooks'] = mod
    antenv.axon_hooks = mod


def run(inputs: dict, trace: bool = False):
    if trace:
        _ensure_ntff_hook()
    nc = _get_program()
    in_maps = _host_prep(**inputs)
    res = run_bass_kernel_spmd(nc, in_maps, core_ids=list(range(NCORES)),
                               trace=trace)
    out = np.concatenate(
        [np.asarray(res.results[c]["out"]).reshape(128, NSUB).T.reshape(BC, 1)
         for c in range(NCORES)]
    )
    return out.astype(np.float32), res


def kernel(**inputs):
    out, _ = run(inputs, trace=False)
    return out


# revision 59
# speedup vs baseline: 1.2966x; 1.2966x over previous
"""DeepCross kernel for 8x TRN2 NeuronCores.

Math: the cross-network keeps temp = x0 * f with f a per-row scalar, so the
whole model collapses to G = x0 @ [cross_w | w1 | wf_x0]  ([B, 37]) plus a
tiny per-row tail:
    g = G[:, :4]; p1 = G[:, 4:36]; q = G[:, 36:37]
    f1 = 1 + g0 + b0; f2 = f1*g1 + b1; f3 = f2*(1+g2) + b2; f4 = f3*g3 + b3
    h1 = relu(f4 * p1); h2 = relu(h1 @ w2); out = sigmoid(h2 @ wf_h + q + bf)

Device strategy (data-parallel over batch, 1024 rows/core):
  - emb table packed to bf16 row-PAIRS [50000, 256B]; dma_gather with
    idx = x//2 stored int16-WRAPPED (values >= 32768 go negative; the HW
    sign-extends and the address wraps 16.77MB below the in_ap base, so the
    table tensor is 2x sized with the high pairs mirrored into the low half
    and in_ap based at +16.77MB).  One descriptor per lookup, 256B each.
  - gathers run on all 4 SWDGE queues (num_swdge_queues=4) so all 4 GPSIMD
    Q7 core-pairs emit descriptors concurrently; Q7 descriptor emission
    (~7ns/lookup/pair) is the kernel's hard bottleneck.  The engine blocks
    on each queue-0 gather until its emission ends, so the FIRST job is
    tiny (2 slots) to let the other queues start immediately (saves a full
    ~30us warmup wave).
  - 1-of-2 sub-row select: one base copy + one predicated copy on fp32
    bitcast views (halves DVE/ACT element counts).
  - PE-transpose 128x128 chunks, accumulate G^T [37, 128] per subtile on PE.
  - One batched tail after all gathers (mid-stream DVE tail ops run ~10x
    slow under SWDGE activity and delay the selects): f-recurrence on DVE,
    h1@w2 on PE, sigmoid on ACT; out_col [128, 8] DMA'd per core and
    untransposed on host.
"""
import sys
sys.path.insert(0, '/opt/trn_rl_repo')
import os
import numpy as np
import ml_dtypes

from concourse import bass, mybir
import concourse.tile as tile
from concourse import bacc, library_config
from concourse.bass_utils import run_bass_kernel_spmd
from concourse.masks import make_identity
from concourse.tile import add_dep_helper

BF16 = ml_dtypes.bfloat16

B, T, E = 8192, 128, 64
V = 100000
D = T * E                 # 8192
L = 4
H1, H2 = 32, 16
NCORES = 8
BC = B // NCORES          # 1024 batch rows per core
NSUB = BC // 128          # 8 subtiles of 128 rows
NPAD = 16                 # trailing dummy idxs (defeat trailing-neg trim)
PE_EL = 128               # bf16 elements per gathered pair row (256 B)
PAIRS = V // 2            # 50000 pair rows
TBL_N = 131072            # table tensor rows (2x 65536 for the wrap trick)
NCHUNK = D // 128         # 64 d-chunks per subtile
NW = L + H1 + 1           # 37 fused weight columns


def _groups():
    """Flat gather-group schedule shared by host prep and program build.
    The very first group is tiny (2 slots): the GPSIMD engine blocks on each
    queue-0 gather until its emission ends, so a small first job lets the
    other three queues' first gathers dispatch almost immediately instead of
    one full emission later.  Subs 0-5: 32-slot groups; subs 6-7: 16-slot
    groups (shorter final waves so the last subtiles' post-processing
    overlaps better)."""
    out = []
    icol = mcol = 0

    def add(sub, slot0, gsz):
        nonlocal icol, mcol
        nidx = 128 * gsz + NPAD
        out.append(dict(sub=sub, slot0=slot0, nslots=gsz,
                        nidx=nidx, nblk=(nidx + 127) // 128,
                        icol=icol, mcol=mcol))
        icol += nidx // 16
        mcol += gsz

    add(0, 0, 2)
    add(0, 2, 30)
    for h in range(1, 4):
        add(0, h * 32, 32)
    for sub in range(1, NSUB):
        for h in range(4):
            add(sub, h * 32, 32)
    return out


GROUPS = _groups()
IDX_COLS = sum(g['nidx'] // 16 for g in GROUPS)
MSK_COLS = sum(g['nslots'] for g in GROUPS)

_PROGRAM = None
KMODE = os.environ.get('KMODE', 'full')
NSWQ = int(os.environ.get('NSWQ', '4'))     # SWDGE queues (1..4)


def _build_program():
    f32 = mybir.dt.float32
    bf16 = mybir.dt.bfloat16
    nc = bacc.Bacc("TRN2", target_bir_lowering=False, debug=False,
                   num_devices=NCORES, dynamic_dma_scratch_size=32768,
                   num_swdge_queues=NSWQ)

    tblq = nc.dram_tensor("tblq", [TBL_N, PE_EL], bf16, kind="ExternalInput")
    xidx = nc.dram_tensor("xidx", [128, IDX_COLS], mybir.dt.int16,
                          kind="ExternalInput")
    xmask = nc.dram_tensor("xmask", [128, MSK_COLS], mybir.dt.uint8,
                           kind="ExternalInput")
    wbd = nc.dram_tensor("wb", [128, NCHUNK * NW], bf16, kind="ExternalInput")
    w2d = nc.dram_tensor("w2m", [H1, H2], f32, kind="ExternalInput")
    # packed tail constants: [cb(4) | b1(32) | b2(16) | bf(1) | w2T(512) | wfh(16)]
    NPK = L + H1 + H2 + 1 + H1 * H2 + H2
    packd = nc.dram_tensor("packv", [1, NPK], f32, kind="ExternalInput")
    outd = nc.dram_tensor("out", [128, NSUB], f32, kind="ExternalOutput")

    tbl_ap = tblq.ap()[TBL_N // 2:TBL_N, :]   # base at +16.77MB

    AF = mybir.ActivationFunctionType
    OP = mybir.AluOpType

    with tile.TileContext(nc) as tc:
        with (
            tc.tile_pool(name="const", bufs=1) as cpool,
            tc.tile_pool(name="io", bufs=3) as iopool,
            tc.tile_pool(name="quad", bufs=8) as qpool,
            tc.tile_pool(name="x0c", bufs=4) as xpool,
            tc.tile_pool(name="xt", bufs=4) as xtpool,
            tc.tile_pool(name="tail", bufs=2) as sp,
            tc.tile_pool(name="ptp", bufs=4, space="PSUM") as ptpool,
            tc.tile_pool(name="pgt", bufs=2, space="PSUM") as pgpool,
            tc.tile_pool(name="pts", bufs=2, space="PSUM") as pspool,
        ):
            # identity builds use gpsimd ops — issue before load_library so
            # they run during the preamble instead of after LOAD_LIB
            ident = cpool.tile([128, 128], bf16)
            make_identity(nc, ident[:])
            ident32 = cpool.tile([128, 128], f32)
            make_identity(nc, ident32[:])
            nc.gpsimd.load_library(library_config.mlp)

            # idx upload split (separate tiles so wave-0 gathers only dep on
            # the small first DMA)
            ACOLS = sum(g['nidx'] // 16 for g in GROUPS[:4])
            idx_a = cpool.tile([128, ACOLS], mybir.dt.int16, tag="idx_a")
            nc.sync.dma_start(out=idx_a[:], in_=xidx.ap()[:, 0:ACOLS])
            idx_b = cpool.tile([128, IDX_COLS - ACOLS], mybir.dt.int16,
                               tag="idx_b")
            nc.sync.dma_start(out=idx_b[:], in_=xidx.ap()[:, ACOLS:])

            def idx_of(gr):
                c0, c1 = gr['icol'], gr['icol'] + gr['nidx'] // 16
                if c1 <= ACOLS:
                    return idx_a[:, c0:c1]
                return idx_b[:, c0 - ACOLS:c1 - ACOLS]
            msk_all = cpool.tile([128, MSK_COLS], mybir.dt.uint8,
                                 tag="msk_all")
            nc.sync.dma_start(out=msk_all[:], in_=xmask.ap())

            wb_t = cpool.tile([128, NCHUNK * NW], bf16)
            nc.sync.dma_start(out=wb_t[:], in_=wbd.ap())
            w2_t = cpool.tile([H1, H2], f32)
            nc.sync.dma_start(out=w2_t[:], in_=w2d.ap())
            # broadcast packed tail constants to all 128 partitions via a
            # k=1 matmul with a ones column.
            pack_t = cpool.tile([1, NPK], f32)
            nc.sync.dma_start(out=pack_t[:], in_=packd.ap())
            ones_r = cpool.tile([1, 128], f32)
            nc.vector.memset(ones_r[:], 1.0)
            packb = cpool.tile([128, NPK], f32)
            packb_mm = None
            for off in range(0, NPK, 512):
                w = min(512, NPK - off)
                pb_p = pspool.tile([128, 512], f32, tag="tps")
                packb_mm = nc.tensor.matmul(out=pb_p[:, 0:w], lhsT=ones_r[:],
                                            rhs=pack_t[0:1, off:off + w],
                                            start=True, stop=True)
                nc.vector.tensor_copy(out=packb[:, off:off + w],
                                      in_=pb_p[:, 0:w])
            prev_tail_pe0 = packb_mm.ins
            cbb = packb[:, 0:L]
            b1b = packb[:, L:L + H1]
            b2b = packb[:, L + H1:L + H1 + H2]
            bfb = packb[:, L + H1 + H2:L + H1 + H2 + 1]
            OW2 = L + H1 + H2 + 1
            w2b = packb[:, OW2:OW2 + H1 * H2]        # w2T j-major [16, 32]
            wfhb = packb[:, OW2 + H1 * H2:NPK]       # [16]
            out_col = None
            if KMODE != "notail":
                out_col = cpool.tile([128, NSUB], f32, tag="out_col")
            gts_all = cpool.tile([NW, BC], f32, tag="gts_all")

            gtt_all = None
            if KMODE != "notail":
                gtt_all = cpool.tile([128, NSUB * NW], f32, tag="gtt_all")

            def gtt_for(s):
                """Transpose G^T [37,128] -> G [128,37] into gtt_all.  PE+ACT
                only, so it is safe to issue mid-stream right after each
                stash (no DVE-under-SWDGE slowdown)."""
                gtt_p = pspool.tile([128, 64], f32, tag="tps")
                nc.tensor.transpose(
                    out=gtt_p[:, 0:NW],
                    in_=gts_all[:, s * 128:(s + 1) * 128],
                    identity=ident32[0:NW, 0:NW])
                nc.scalar.copy(out=gtt_all[:, s * NW:(s + 1) * NW],
                               in_=gtt_p[:, 0:NW])

            def tail_half(s0, ns):
                """Tail math for subtiles [s0, s0+ns): f-recurrence on DVE,
                h1@w2 on PE, sigmoid into out_col[:, s0:s0+ns]."""
                gv = gtt_all[:, s0 * NW:(s0 + ns) * NW].rearrange(
                    "p (s w) -> p s w", w=NW)

                f1 = cpool.tile([128, ns], f32, tag=f"f1_{s0}")
                nc.vector.tensor_scalar(out=f1[:], in0=gv[:, :, 0:1],
                                        scalar1=cbb[:, 0:1], scalar2=1.0,
                                        op0=OP.add, op1=OP.add)
                f2 = cpool.tile([128, ns], f32, tag=f"f2_{s0}")
                nc.vector.tensor_tensor(out=f2[:], in0=f1[:].rearrange(
                    "p (s one) -> p s one", one=1), in1=gv[:, :, 1:2],
                    op=OP.mult)
                nc.vector.tensor_scalar(out=f2[:], in0=f2[:],
                                        scalar1=cbb[:, 1:2], scalar2=None,
                                        op0=OP.add)
                u3 = cpool.tile([128, ns], f32, tag=f"u3_{s0}")
                nc.vector.tensor_scalar(out=u3[:], in0=gv[:, :, 2:3],
                                        scalar1=1.0, scalar2=None, op0=OP.add)
                f3 = cpool.tile([128, ns], f32, tag=f"f3_{s0}")
                nc.vector.tensor_tensor(out=f3[:], in0=f2[:], in1=u3[:],
                                        op=OP.mult)
                nc.vector.tensor_scalar(out=f3[:], in0=f3[:],
                                        scalar1=cbb[:, 2:3], scalar2=None,
                                        op0=OP.add)
                f4 = cpool.tile([128, ns], f32, tag=f"f4_{s0}")
                nc.vector.tensor_tensor(out=f4[:], in0=f3[:].rearrange(
                    "p (s one) -> p s one", one=1), in1=gv[:, :, 3:4],
                    op=OP.mult)
                nc.vector.tensor_scalar(out=f4[:], in0=f4[:],
                                        scalar1=cbb[:, 3:4], scalar2=None,
                                        op0=OP.add)

                # h1 = relu(f4 * p1 + b1)  [128, ns, H1]
                h1_all = cpool.tile([128, ns * H1], f32, tag=f"h1_{s0}")
                h1v = h1_all[:].rearrange("p (s h) -> p s h", h=H1)
                nc.vector.tensor_tensor(
                    out=h1v, in0=gv[:, :, L:L + H1],
                    in1=f4[:].rearrange("p (s one) -> p s one", one=1)
                        .to_broadcast([128, ns, H1]),
                    op=OP.mult)
                nc.vector.tensor_tensor(
                    out=h1v, in0=h1v,
                    in1=b1b.rearrange("p (one h) -> p one h", one=1)
                        .to_broadcast([128, ns, H1]),
                    op=OP.add)
                nc.scalar.activation(out=h1_all[:], in_=h1_all[:], func=AF.Relu)

                # h2 = relu(h1 @ w2 + b2) via PE (idle at tail time):
                # transpose h1 into (s,h)-on-partitions halves, 4 subs each
                h2_all = cpool.tile([128, ns * H2], f32, tag=f"h2_{s0}")
                h2v = h2_all[:].rearrange("p (s j) -> p s j", j=H2)
                for q4 in range((ns + 3) // 4):
                    nsq = min(4, ns - q4 * 4)
                    h1T_p = pspool.tile([128, 128], f32, tag="tps")
                    nc.tensor.transpose(
                        out=h1T_p[0:nsq * H1, :],
                        in_=h1_all[:, q4 * 4 * H1:(q4 * 4 + nsq) * H1],
                        identity=ident32[:])
                    h1T = cpool.tile([128, 128], f32, tag=f"h1T_{s0}_{q4}")
                    nc.scalar.copy(out=h1T[0:nsq * H1, :],
                                   in_=h1T_p[0:nsq * H1, :])
                    for j in range(nsq):
                        sl = q4 * 4 + j
                        h1Tj = cpool.tile([H1, 128], f32,
                                          tag=f"h1Tj_{s0}_{sl}")
                        nc.scalar.copy(out=h1Tj[:],
                                       in_=h1T[j * H1:(j + 1) * H1, :])
                        h2p = pspool.tile([128, H2], f32, tag="tps")
                        nc.tensor.matmul(out=h2p[:], lhsT=h1Tj[:],
                                         rhs=w2_t[:], start=True, stop=True)
                        nc.scalar.copy(out=h2_all[:, sl * H2:(sl + 1) * H2],
                                       in_=h2p[:])
                nc.vector.tensor_tensor(
                    out=h2v, in0=h2v,
                    in1=b2b.rearrange("p (one h) -> p one h", one=1)
                        .to_broadcast([128, ns, H2]),
                    op=OP.add)
                nc.scalar.activation(out=h2_all[:], in_=h2_all[:], func=AF.Relu)

                # z = h2 @ wf_h ; out = sigmoid(z + q + bf)
                zt = cpool.tile([128, ns * H2], f32, tag=f"zt_{s0}")
                ztv = zt[:].rearrange("p (s j) -> p s j", j=H2)
                nc.vector.tensor_tensor(
                    out=ztv, in0=h2v,
                    in1=wfhb.rearrange("p (one j) -> p one j", one=1)
                        .to_broadcast([128, ns, H2]),
                    op=OP.mult)
                z_all = cpool.tile([128, ns], f32, tag=f"z_{s0}")
                nc.vector.tensor_reduce(
                    out=z_all[:].rearrange("p (s one) -> p s one", one=1),
                    in_=ztv, axis=mybir.AxisListType.X, op=OP.add)
                nc.vector.tensor_tensor(out=z_all[:], in0=z_all[:].rearrange(
                    "p (s one) -> p s one", one=1), in1=gv[:, :, NW - 1:NW],
                    op=OP.add)
                nc.scalar.activation(out=out_col[:, s0:s0 + ns], in_=z_all[:],
                                     func=AF.Sigmoid,
                                     bias=bfb[:, 0:1], scale=1.0)

            prev_tail_pe = prev_tail_pe0
            MAXBLK = max(g['nblk'] for g in GROUPS)
            gt = None
            cur_sub = -1
            for jidx, gr in enumerate(GROUPS):
                sub, slot0, nsl = gr['sub'], gr['slot0'], gr['nslots']
                if sub != cur_sub:
                    gt = pgpool.tile([NW, 128], f32, tag="gt",
                                     name=f"gt{sub}")
                    cur_sub = sub
                idx_t = idx_of(gr)
                msk_t = msk_all[:, gr['mcol']:gr['mcol'] + nsl]

                quad = qpool.tile([128, MAXBLK * PE_EL], bf16, tag="quad")
                qview = quad[:].rearrange("p (s e) -> p s e", e=PE_EL)
                nc.gpsimd.dma_gather(
                    out_ap=qview[:, 0:gr['nblk'], :],
                    in_ap=tbl_ap,
                    idxs_ap=idx_t[:],
                    num_idxs=gr['nidx'],
                    num_idxs_reg=gr['nidx'],
                    elem_size=PE_EL,
                    single_packet=False,
                    queue_num=jidx % NSWQ,
                )

                x0c = xpool.tile([128, 32 * 64], bf16, tag="x0c")
                qv32 = quad[:].bitcast(f32).rearrange("p (s e) -> p s e",
                                                      e=PE_EL // 2)
                xv32 = x0c[:].bitcast(f32).rearrange("p (s e) -> p s e",
                                                     e=32)
                HG = nsl // 2

                def _mk(lo, hi):
                    m = msk_t[:, lo:hi]
                    m = m.rearrange("p (s one) -> p s one", one=1)
                    return m.to_broadcast([128, hi - lo, 32])

                for lo in (0, HG):
                    hi = lo + HG
                    nc.scalar.copy(out=xv32[:, lo:hi, :],
                                   in_=qv32[:, lo:hi, 0:32])
                    nc.vector.copy_predicated(out=xv32[:, lo:hi, :],
                                              mask=_mk(lo, hi),
                                              data=qv32[:, lo:hi, 32:64])
                chunks = nsl // 2
                for c0 in range(0, chunks, 4):
                    blkn = min(4, chunks - c0)
                    tp = ptpool.tile([128, 512], bf16, tag="tp")
                    for j in range(blkn):
                        c2 = c0 + j
                        nc.tensor.transpose(
                            out=tp[:, j * 128:(j + 1) * 128],
                            in_=x0c[:, c2 * 128:(c2 + 1) * 128],
                            identity=ident[:],
                        )
                    xt = xtpool.tile([128, 512], bf16, tag="xt")
                    nc.scalar.copy(out=xt[:, 0:blkn * 128],
                                   in_=tp[:, 0:blkn * 128])
                    for j in range(blkn):
                        cd = slot0 // 2 + c0 + j
                        mm = nc.tensor.matmul(
                            out=gt[:],
                            lhsT=wb_t[:, cd * NW:(cd + 1) * NW],
                            rhs=xt[:, j * 128:(j + 1) * 128],
                            start=(cd == 0),
                            stop=(cd == NCHUNK - 1),
                        )
                        if cd == 0 and sub == 0 and prev_tail_pe is not None:
                            add_dep_helper(mm.ins, prev_tail_pe,
                                           reason="packb before accum groups")

                if slot0 + nsl == 128:
                    # stash G^T for the batched tail
                    nc.scalar.copy(out=gts_all[:, sub * 128:(sub + 1) * 128],
                                   in_=gt[:])
                    if KMODE != "notail":
                        gtt_for(sub)

            if KMODE == "notail":
                nc.sync.dma_start(out=outd.ap(), in_=wb_t[:, 0:NSUB])
            else:
                # single post-loop tail: mid-stream DVE injections run ~10x
                # slow under SWDGE activity and delay the selects
                tail_half(0, NSUB)
                nc.sync.dma_start(out=outd.ap(), in_=out_col[:])

    nc.compile()
    return nc


def _get_program():
    global _PROGRAM
    if _PROGRAM is None:
        _PROGRAM = _build_program()
    return _PROGRAM


def _host_prep(x, emb, cross_w, cross_b, w1, b1, w2, b2, wf, bf):
    x = np.asarray(x)
    emb = np.ascontiguousarray(np.asarray(emb, dtype=np.float32))
    cross_w = np.asarray(cross_w, dtype=np.float32)
    cross_b = np.asarray(cross_b, dtype=np.float32)
    w1 = np.asarray(w1, dtype=np.float32)
    w2 = np.asarray(w2, dtype=np.float32)
    b1 = np.asarray(b1, dtype=np.float32)
    b2 = np.asarray(b2, dtype=np.float32)
    wf = np.asarray(wf, dtype=np.float32)
    bf = np.asarray(bf, dtype=np.float32)

    # pair table with the int16-wrap mirror: real pairs at [65536, 65536+50000),
    # pairs >= 32768 mirrored at their raw index for wrapped (negative) idxs.
    pe = emb.astype(BF16).reshape(PAIRS, PE_EL)
    tbl = np.zeros((TBL_N, PE_EL), dtype=BF16)
    tbl[TBL_N // 2:TBL_N // 2 + PAIRS] = pe
    tbl[32768:PAIRS] = pe[32768:PAIRS]

    wbig = np.concatenate([cross_w[:, :, 0].T, w1, wf[H2:, :]], axis=1)  # [D, 37]
    wb_np = np.ascontiguousarray(
        wbig.reshape(NCHUNK, 128, NW).transpose(1, 0, 2).reshape(128, NCHUNK * NW)
    ).astype(BF16)

    # [cb(4) | b1(32) | b2(16) | bf(1) | w2T j-major (512) | wfh (16)]
    packv = np.concatenate([
        cross_b.reshape(-1), b1.reshape(-1), b2.reshape(-1), bf.reshape(-1),
        w2.T.reshape(-1), wf[:H2, 0].reshape(-1),
    ]).astype(np.float32).reshape(1, -1)

    shared = {
        "tblq": tbl,
        "wb": wb_np,
        "packv": packv,
        "w2m": w2,
    }

    in_maps = []
    for c in range(NCORES):
        xc = x[c * BC:(c + 1) * BC].astype(np.int64)
        xq = (xc // 2).astype(np.int32)           # pair idx, wraps to int16
        xr = (xc % 2).astype(np.uint8)
        idx_np = np.empty((128, IDX_COLS), dtype=np.int16)
        msk_np = np.empty((128, MSK_COLS), dtype=np.uint8)
        for gr in GROUPS:
            s, slot0, nsl = gr['sub'], gr['slot0'], gr['nslots']
            blk = xq[s * 128:(s + 1) * 128, slot0:slot0 + nsl]  # [128b, nsl]
            lst = np.concatenate([blk.T.reshape(-1),            # i = t*128+b
                                  np.zeros(NPAD, dtype=np.int32)])
            cw = gr['nidx'] // 16
            idx_np[:, gr['icol']:gr['icol'] + cw] = np.tile(
                lst.reshape(cw, 16).T.astype(np.int16), (8, 1))
            msk_np[:, gr['mcol']:gr['mcol'] + nsl] = \
                xr[s * 128:(s + 1) * 128, slot0:slot0 + nsl]
        m = dict(shared)
        m["xidx"] = idx_np
        m["xmask"] = msk_np
        in_maps.append(m)
    return in_maps


def _ensure_ntff_hook():
    """The image's antenv lacks axon_hooks; synthesize it so
    run_bass_kernel_spmd(trace=True) can NTFF-profile via the axon .so."""
    import types
    if 'antenv.axon_hooks' in sys.modules:
        return
    import antenv
    mod = types.ModuleType('antenv.axon_hooks')
    _state = {'hook': None}
    def set_axon_ntff_profile_hook(h):
        _state['hook'] = h
    def get_axon_ntff_profile_hook():
        if _state['hook'] is None:
            try:
                from trn_agent_boot.trn_boot import _ntff_profile_via_ctypes
                _state['hook'] = _ntff_profile_via_ctypes('/opt/axon/libaxon_pjrt.so')
            except Exception:
                return None
        return _state['hook']
    mod.set_axon_ntff_profile_hook = set_axon_ntff_profile_hook
    mod.get_axon_ntff_profile_hook = get_axon_ntff_profile_hook
    sys.modules['antenv.axon_hooks'] = mod
    antenv.axon_hooks = mod


def run(inputs: dict, trace: bool = False):
    if trace:
        _ensure_ntff_hook()
    nc = _get_program()
    in_maps = _host_prep(**inputs)
    res = run_bass_kernel_spmd(nc, in_maps, core_ids=list(range(NCORES)),
                               trace=trace)
    out = np.concatenate(
        [np.asarray(res.results[c]["out"]).reshape(128, NSUB).T.reshape(BC, 1)
         for c in range(NCORES)]
    )
    return out.astype(np.float32), res


def kernel(**inputs):
    out, _ = run(inputs, trace=False)
    return out
